# revision 16
# baseline (speedup 1.0000x reference)
"""Trainium2 Bass kernel for nn_Attn_head (GAT attention head, B=1) — v13.

v12 -> v13 (same math, scheduling/overlap):
  - wf computed host-side (already needed for fmax); w1T/wf/ep packed
    into ONE input tensor "win" -> 3 fewer serial DMA_DIRECT2D issues.
  - xs chunk-0 DMA issued first; xI issued after chunk 1.
  - PE warm-keeping dummy matmuls in the three idle windows (startup,
    Ue->dUT handoff, mid-tail) so the HAM clock gate stays at 8/8.
  - chunk-loop engine rebalance: second sft/F_ALL copies on DVE, nf on
    GpSimd; ACT keeps one copy of each + the exps.
  - tail software-pipelined in 4 supersets of 16 j-blocks
    (U3 copy -> D math -> gall -> TG accumulate), U3 matmuls emitted
    one superset ahead so the PE queue never drains.
"""

import sys
import numpy as np

for _p in ("/opt/trn_rl_repo", "/root/.axon_site/_ro/trn_rl_repo"):
    if _p not in sys.path:
        sys.path.insert(0, _p)

import concourse.bacc as bacc
import concourse.bass as bass
import concourse.mybir as mybir
import concourse.tile as tile
import concourse.masks as masks
import ml_dtypes
from concourse.bass_utils import run_bass_kernel_spmd

FP32 = mybir.dt.float32
BF16 = mybir.dt.bfloat16
INT32 = mybir.dt.int32
ALU = mybir.AluOpType
AF = mybir.ActivationFunctionType

CIN = 128
COUT = 64
W = COUT + 1  # sft width per j-block: seq_ftsT cols + (-f) col
JBW = 128     # j-block width (PE contraction tile)
MF = 512      # moving free dim per matmul (one PSUM bank of fp32)
XCH = 1024    # x staging chunk (columns per DMA)
PBB = 4       # preamble j-blocks per PSUM tile
NE = 128      # grid size (one partition tile)
NEX = NE + 8  # +8 sentinel always-true cols (col NE => column sums)
WIN = COUT + 5  # packed win cols: w1T | wfT | ep(4, on row 0)
SCJ = 32      # tail superset size (j-blocks)


def build(N=8192, CORES=8):
    nc = bacc.Bacc("TRN2", target_bir_lowering=False, debug=False,
                   num_devices=CORES)
    IC = N // CORES
    x_d = nc.dram_tensor("x", [CIN, N], BF16, kind="ExternalInput")
    xI_d = nc.dram_tensor("xI", [CIN, IC], BF16, kind="ExternalInput")
    win_d = nc.dram_tensor("win", [CIN, WIN], FP32, kind="ExternalInput")
    y_d = nc.dram_tensor("y", [COUT, IC], FP32, kind="ExternalOutput")

    with tile.TileContext(nc) as tc:
        _build_body(tc, nc, x_d, xI_d, win_d, y_d, N, CORES)
    nc.compile()
    return nc


def _build_body(tc, nc, x_d, xI_d, win_d, y_d, N, CORES):
    from contextlib import ExitStack
    IC = N // CORES
    NJB = N // JBW
    NCH = N // XCH
    CPX = XCH // JBW
    NH = max(IC // MF, 1)
    MFi = min(MF, IC)
    NSC = max(NJB // SCJ, 1)
    SCJi = min(SCJ, NJB)

    ctx = ExitStack()
    with ctx:
        sb = ctx.enter_context(tc.tile_pool(name="sb", bufs=1))
        xpool = ctx.enter_context(tc.tile_pool(name="xpool", bufs=3))
        mbpool = ctx.enter_context(tc.tile_pool(name="mbpool", bufs=1))
        m1pool = ctx.enter_context(tc.tile_pool(name="m1pool", bufs=1))
        eppool = ctx.enter_context(tc.tile_pool(name="eppool", bufs=2))
        ue_ps_pool = ctx.enter_context(
            tc.tile_pool(name="ue_ps", bufs=1, space="PSUM"))
        pre_ps_pool = ctx.enter_context(
            tc.tile_pool(name="pre_ps", bufs=1, space="PSUM"))
        wp_ps_pool = ctx.enter_context(
            tc.tile_pool(name="wp_ps", bufs=1, space="PSUM"))
        fa_ps_pool = ctx.enter_context(
            tc.tile_pool(name="fa_ps", bufs=2, space="PSUM"))
        big_ps_pool = ctx.enter_context(
            tc.tile_pool(name="big_ps", bufs=1, space="PSUM"))
        misc_ps_pool = ctx.enter_context(
            tc.tile_pool(name="misc_ps", bufs=1, space="PSUM"))

        # ---------------- DMA issue order: x chunk 0 first ----------------
        xs_tiles = []
        xs0 = xpool.tile([CIN, XCH], BF16, name="xs0", tag="xs")
        nc.sync.dma_start(xs0[:, :], x_d.ap()[:, 0:XCH])
        xs_tiles.append(xs0)
        win = sb.tile([CIN, WIN], FP32)
        nc.sync.dma_start(win[:, :], win_d.ap())
        if NCH > 1:
            xs1 = xpool.tile([CIN, XCH], BF16, name="xs1", tag="xs")
            nc.sync.dma_start(xs1[:, :], x_d.ap()[:, XCH:2 * XCH])
            xs_tiles.append(xs1)
        xI_sb = xpool.tile([CIN, IC], BF16, name="xI", tag="xi")
        nc.sync.dma_start(xI_sb[:, :], xI_d.ap())

        # ---------------- phase 0: constants & grid ----------------
        ones = sb.tile([128, 128], FP32)
        nc.gpsimd.memset(ones[:, :], 1.0)
        idT = sb.tile([128, 128], BF16)
        masks.make_identity(nc, idT[:, :])
        id2 = sb.tile([128, COUT], BF16)   # stacked double identity
        masks.make_identity(nc, id2[0:COUT, :])
        masks.make_identity(nc, id2[COUT:128, :])
        it_e = sb.tile([128, NE], INT32)
        nc.gpsimd.iota(it_e[:, :], pattern=[[1, NE]], channel_multiplier=0)
        it_p = sb.tile([128, 1], INT32)
        nc.gpsimd.iota(it_p[:, :], pattern=[[128, 1]], channel_multiplier=1)

        # PE warm-up: keep the HAM activity window busy before real work.
        # wp is a dedicated never-recycled PSUM tile so dummies can be
        # sprinkled anywhere without aliasing live accumulations.
        wp = wp_ps_pool.tile([128, 128], FP32, name="wp", tag="wp")
        for wv in range(44):
            nc.tensor.matmul(wp[:, :], idT[:, :], idT[:, :])

        wfull = sb.tile([CIN, W], BF16)
        nc.vector.tensor_copy(wfull[:, 0:COUT], win[:, 0:COUT])
        nc.vector.tensor_scalar(wfull[:, COUT:W], win[:, COUT:COUT + 1],
                                -1.0, None, ALU.mult)
        wf_rep = sb.tile([CIN, 128], BF16)
        nc.vector.tensor_scalar(wf_rep[:, :], ones[:, :],
                                win[:, COUT:COUT + 1], None, ALU.mult)

        epb_ps = misc_ps_pool.tile([128, 4], FP32, name="epb_ps", tag="mm")
        nc.tensor.matmul(epb_ps[:, :], ones[0:1, :], win[0:1, COUT + 1:WIN])
        epb = sb.tile([128, 4], FP32)
        nc.scalar.activation(epb[:, :], epb_ps[:, :], AF.Copy)
        E_bc = sb.tile([128, NEX], BF16)   # E_bc[p,t] = e_t; cols NE.. = -inf
        nc.scalar.activation(E_bc[:, 0:NE], it_e[:, :], AF.Identity,
                             bias=epb[:, 0:1], scale=epb[:, 1:2])
        nc.gpsimd.memset(E_bc[:, NE:NEX], -1.0e38)
        ecol = sb.tile([128, 1], FP32)     # ecol[p] = -e_p
        nc.scalar.activation(ecol[:, :], it_p[:, :], AF.Identity,
                             bias=epb[:, 2:3], scale=epb[:, 3:4])

        # ---------------- F broadcast (own i-shard) ----------------
        F_sb = sb.tile([128, IC], BF16)    # f[i] bcast over partitions
        ab_bc = sb.tile([128, IC], FP32)   # rows 0:64 exp(f), 64:128 exp(.01f)
        for h in range(NH):
            sl = slice(h * MFi, (h + 1) * MFi)
            fi_ps = fa_ps_pool.tile([128, MFi], FP32, name=f"fi{h}", tag="fa")
            nc.tensor.matmul(fi_ps[:, :], wf_rep[:, :], xI_sb[:, sl])
            nc.scalar.activation(F_sb[:, sl], fi_ps[:, :], AF.Copy)
            nc.scalar.activation(ab_bc[0:COUT, sl], fi_ps[0:COUT, :], AF.Exp)
            nc.scalar.activation(ab_bc[COUT:128, sl], fi_ps[COUT:128, :],
                                 AF.Exp, scale=0.01)
        mt2 = sb.tile([128, IC], BF16)     # mt2[p,i] = [e_p <= -f_i]
        nc.vector.tensor_scalar(mt2[:, :], F_sb[:, :], ecol[:, 0:1], None,
                                ALU.is_le)

        # ---------------- preamble chunk loop ----------------
        sft = sb.tile([128, NJB * W], BF16)   # [j_in_block, (JB, o|-f)]
        F_ALL = sb.tile([128, N], BF16)       # f[j] bcast over partitions
        nf = sb.tile([128, NJB], FP32)        # -f[j]
        a_all = sb.tile([128, NJB], FP32)
        b_all = sb.tile([128, NJB], FP32)
        pf = sb.tile([128, NJB], FP32)        # +f[j]
        abp = sb.tile([128, 2 * NJB], BF16)   # per jb: [b, a] lhsT cols
        Ue_ps = ue_ps_pool.tile([2, NEX], FP32, name="Ue_ps", tag="ue")
        mb_tiles = []
        m1_tiles = []
        for ch in range(NCH):
            j0 = ch * CPX
            if ch < len(xs_tiles):
                xs = xs_tiles[ch]
            else:
                xs = xpool.tile([CIN, XCH], BF16, name=f"xs{ch}", tag="xs")
                nc.sync.dma_start(xs[:, :],
                                  x_d.ap()[:, ch * XCH:(ch + 1) * XCH])
            # F_ALL chunk: stationary wf_rep, one matmul per 512 cols
            for g in range(XCH // MF):
                fa = fa_ps_pool.tile([128, MF], FP32, name=f"fa{ch}{g}",
                                     tag="fa")
                nc.tensor.matmul(fa[:, :], wf_rep[:, :],
                                 xs[:, g * MF:(g + 1) * MF])
                dst = F_ALL[:, ch * XCH + g * MF:ch * XCH + (g + 1) * MF]
                if g == 0:
                    nc.scalar.activation(dst, fa[:, :], AF.Copy)
                else:
                    nc.vector.tensor_copy(dst, fa[:, :])
            # m1 chunk: m1[p, j] = [e_p <= -f_j]
            m1c = m1pool.tile([128, XCH], BF16, name=f"m1_{ch}", tag=f"m1{ch}")
            nc.vector.tensor_scalar(
                m1c[:, :], F_ALL[:, ch * XCH:(ch + 1) * XCH], ecol[:, 0:1],
                None, ALU.is_le)
            m1_tiles.append(m1c)
            # seq_ftsT + (-f) per PBB-group
            for g in range(CPX // PBB):
                jg = j0 + g * PBB
                pre = pre_ps_pool.tile([128, PBB * W], FP32, name=f"pre{jg}",
                                       tag="pre")
                for k in range(PBB):
                    xo = (g * PBB + k) * JBW
                    nc.tensor.matmul(pre[:, k * W:(k + 1) * W],
                                     xs[:, xo:xo + JBW], wfull[:, :])
                dst = sft[:, jg * W:(jg + PBB) * W]
                if g == 0:
                    nc.scalar.activation(dst, pre[:, :], AF.Copy)
                else:
                    nc.vector.tensor_copy(dst, pre[:, :])
            csl = slice(j0, j0 + CPX)
            nc.scalar.activation(
                nf[:, csl], sft[:, j0 * W + COUT:(j0 + CPX) * W:W], AF.Copy)
            nc.scalar.activation(a_all[:, csl], nf[:, csl], AF.Exp, scale=-1.0)
            nc.scalar.activation(b_all[:, csl], nf[:, csl], AF.Exp,
                                 scale=-0.01)
            nc.gpsimd.tensor_scalar(pf[:, csl], nf[:, csl], -1.0, None,
                                    ALU.mult)
            nc.gpsimd.tensor_copy(abp[:, 2 * j0 + 0:2 * (j0 + CPX):2],
                                  b_all[:, csl])
            nc.gpsimd.tensor_copy(abp[:, 2 * j0 + 1:2 * (j0 + CPX):2],
                                  a_all[:, csl])
            for jb in range(j0, j0 + CPX):
                mb = mbpool.tile([128, NEX], BF16, name=f"mb{jb}",
                                 tag=f"mb{jb}")
                eng = nc.gpsimd if (jb - j0) >= CPX - 2 else nc.vector
                eng.tensor_scalar(mb[:, :], E_bc[:, :],
                                  pf[:, jb:jb + 1], None, ALU.is_le)
                nc.tensor.matmul(Ue_ps[:, :], abp[:, 2 * jb:2 * jb + 2],
                                 mb[:, :],
                                 start=(jb == 0), stop=(jb == NJB - 1))
                mb_tiles.append(mb)
            for wv in range(2):
                nc.tensor.matmul(wp[:, :], idT[:, :], idT[:, :])

        # ---------------- U_e -> dU; PE kept warm over the handoff --------
        Ue = sb.tile([2, NEX], FP32)
        nc.scalar.activation(Ue[:, :], Ue_ps[:, :], AF.Copy)
        for wv in range(10):
            nc.tensor.matmul(wp[:, :], idT[:, :], idT[:, :])
        sb_ps = misc_ps_pool.tile([128, 1], FP32, name="sb_ps", tag="mm")
        nc.tensor.matmul(sb_ps[:, :], ones[0:1, :], Ue[0:1, 0:1])
        Sb_bc = sb.tile([128, 1], FP32)
        nc.scalar.activation(Sb_bc[:, :], sb_ps[:, :], AF.Copy)
        dU = sb.tile([2, NE], BF16)
        nc.vector.tensor_tensor(dU[:, 1:NE - 1], Ue[:, 2:NE], Ue[:, 1:NE - 1],
                                ALU.subtract)
        nc.vector.tensor_scalar(dU[:, NE - 1:NE], Ue[:, NE - 1:NE], -1.0,
                                None, ALU.mult)
        nc.vector.tensor_scalar(dU[:, 0:1], Ue[:, 1:2], 1.0, None, ALU.mult)
        dUT_ps = misc_ps_pool.tile([128, 2], BF16, name="dUT_ps", tag="mm")
        nc.tensor.transpose(dUT_ps[:, :], dU[:, :], idT[0:2, 0:2])
        dUT = sb.tile([128, 2], BF16)
        nc.scalar.activation(dUT[:, :], dUT_ps[:, :], AF.Copy)

        # ------- tail: supersets of SCJ j-blocks, pipelined by one -------
        U3_ps = misc_ps_pool.tile([128, 2 * NJB], FP32, name="U3", tag="mm")
        U3 = sb.tile([128, 2 * NJB], FP32)
        aDb = sb.tile([128, NJB], FP32)
        bDb = sb.tile([128, NJB], FP32)
        t1 = sb.tile([128, NJB], FP32)
        s2 = sb.tile([128, NJB], FP32)
        t2 = sb.tile([128, NJB], FP32)
        D_T = sb.tile([128, NJB], FP32)
        Dinv = sb.tile([128, NJB], FP32)
        TG_ps = ue_ps_pool.tile([128, NEX], FP32, name="TG_ps", tag="ue")
        gtiles = {}

        def emit_u3(sc):
            for jb in range(sc * SCJi, (sc + 1) * SCJi):
                nc.tensor.matmul(
                    U3_ps[:, 2 * jb:2 * jb + 2],
                    m1_tiles[jb // CPX][:, (jb % CPX) * JBW:
                                        (jb % CPX + 1) * JBW],
                    dUT[:, :])

        def emit_dgal(sc):
            jsl = slice(sc * SCJi, (sc + 1) * SCJi)
            u2sl = slice(2 * sc * SCJi, 2 * (sc + 1) * SCJi)
            nc.scalar.activation(U3[:, u2sl], U3_ps[:, u2sl], AF.Copy)
            for wv in range(4):
                nc.tensor.matmul(wp[:, :], idT[:, :], idT[:, :])
            nc.vector.tensor_tensor(t1[:, jsl], a_all[:, jsl],
                                    U3[:, u2sl][:, 1::2], ALU.mult)
            nc.vector.tensor_scalar(s2[:, jsl], U3[:, u2sl][:, 0::2],
                                    Sb_bc[:, 0:1], -1.0,
                                    ALU.subtract, ALU.mult)
            nc.vector.tensor_tensor(t2[:, jsl], b_all[:, jsl], s2[:, jsl],
                                    ALU.mult)
            nc.vector.tensor_tensor(D_T[:, jsl], t1[:, jsl], t2[:, jsl],
                                    ALU.add)
            nc.vector.reciprocal(Dinv[:, jsl], D_T[:, jsl])
            nc.vector.tensor_tensor(aDb[:, jsl], a_all[:, jsl], Dinv[:, jsl],
                                    ALU.mult)
            nc.vector.tensor_tensor(bDb[:, jsl], b_all[:, jsl], Dinv[:, jsl],
                                    ALU.mult)
            for c in range(sc * SCJi // CPX,
                           max((sc + 1) * SCJi // CPX, sc * SCJi // CPX + 1)):
                j0 = c * CPX
                gpx = min(CPX, NJB - j0)
                gc = sb.tile([128, gpx * 2 * COUT], BF16, name=f"gall{c}")
                gvv = gc[:, :].rearrange("p (j t) -> p j t", t=2 * COUT)
                sfv = sft[:, j0 * W:(j0 + gpx) * W].rearrange(
                    "p (j w) -> p j w", w=W)[:, :, 0:COUT]
                nc.vector.tensor_tensor(
                    gvv[:, :, 0:COUT], sfv,
                    aDb[:, j0:j0 + gpx].unsqueeze(2).broadcast_to(
                        [128, gpx, COUT]), ALU.mult)
                nc.gpsimd.tensor_tensor(
                    gvv[:, :, COUT:2 * COUT], sfv,
                    bDb[:, j0:j0 + gpx].unsqueeze(2).broadcast_to(
                        [128, gpx, COUT]), ALU.mult)
                gtiles[c] = gc

        def emit_tg(sc):
            for jb in range(sc * SCJi, (sc + 1) * SCJi):
                gc = gtiles[jb // CPX]
                go = (jb % CPX) * 2 * COUT
                nc.tensor.matmul(TG_ps[:, :], gc[:, go:go + 2 * COUT],
                                 mb_tiles[jb][:, :],
                                 start=(jb == 0), stop=(jb == NJB - 1))

        emit_u3(0)
        if NSC > 1:
            emit_dgal(0)
            emit_u3(1)
            for sc in range(2, NSC):
                emit_tg(sc - 2)
                emit_dgal(sc - 1)
                emit_u3(sc)
            emit_tg(NSC - 2)
            emit_dgal(NSC - 1)
            emit_tg(NSC - 1)
        else:
            emit_dgal(0)
            emit_tg(0)

        sgb_col = sb.tile([128, 1], FP32)   # rows 64:128 = SGb
        nc.scalar.activation(sgb_col[COUT:128, :], TG_ps[COUT:128, NE:NE + 1],
                             AF.Copy)
        TGs = sb.tile([128, NEX], FP32)
        nc.scalar.activation(TGs[:, :], TG_ps[:, :], AF.Copy)
        for wv in range(6):
            nc.tensor.matmul(wp[:, :], idT[:, :], idT[:, :])
        dTG = sb.tile([128, NE], BF16)
        nc.vector.tensor_tensor(dTG[:, 1:NE - 1], TGs[:, 2:NE],
                                TGs[:, 1:NE - 1], ALU.subtract)
        nc.vector.tensor_scalar(dTG[:, NE - 1:NE], TGs[:, NE - 1:NE], -1.0,
                                None, ALU.mult)
        nc.vector.tensor_scalar(dTG[:, 0:1], TGs[:, 1:2], 1.0, None, ALU.mult)
        dTGT_ps = misc_ps_pool.tile([128, 128], BF16, name="dTGT_ps", tag="mm")
        nc.tensor.transpose(dTGT_ps[:, :], dTG[:, :], idT[:, :])
        dTGT = sb.tile([128, 128], BF16)
        nc.scalar.activation(dTGT[:, :], dTGT_ps[:, :], AF.Copy)

        # ---------------- gather + epilogue per column half ----------------
        out_ps = big_ps_pool.tile([128, IC], FP32, name="out_ps", tag="big")
        for h2 in range(NH):
            sl2 = slice(h2 * MFi, (h2 + 1) * MFi)
            nc.tensor.matmul(out_ps[:, sl2], dTGT[:, :], mt2[:, sl2])
            tfu = eppool.tile([128, MFi], BF16, name=f"tf{h2}", tag="e1")
            nc.vector.tensor_tensor(tfu[0:COUT, :], ab_bc[0:COUT, sl2],
                                    out_ps[0:COUT, sl2], ALU.mult)
            eb = eppool.tile([128, MFi], FP32, name=f"eb{h2}", tag="e2")
            nc.scalar.activation(eb[COUT:128, :], out_ps[COUT:128, sl2],
                                 AF.Identity, bias=sgb_col[COUT:128, 0:1],
                                 scale=-1.0)
            nc.vector.tensor_tensor(tfu[COUT:128, :], ab_bc[COUT:128, sl2],
                                    eb[COUT:128, :], ALU.mult)
            z_ps = misc_ps_pool.tile([COUT, MFi], FP32, name=f"z{h2}",
                                     tag="mm")
            nc.tensor.matmul(z_ps[:, :], id2[:, :], tfu[:, :])
            e = eppool.tile([COUT, MFi], BF16, name=f"e{h2}", tag="e3")
            nc.scalar.activation(e[:, :], z_ps[:, :], AF.Exp)
            r = eppool.tile([COUT, MFi], BF16, name=f"r{h2}", tag="e4")
            nc.scalar.activation(r[:, :], z_ps[:, :], AF.Relu)
            q = eppool.tile([COUT, MFi], BF16, name=f"q{h2}", tag="e5")
            nc.vector.tensor_scalar(q[:, :], e[:, :], 1.0, -1.0, ALU.min,
                                    ALU.add)
            y_sb = eppool.tile([COUT, MFi], FP32, name=f"y{h2}", tag="e6")
            nc.vector.tensor_tensor(y_sb[:, :], r[:, :], q[:, :], ALU.add)
            nc.sync.dma_start(y_d.ap()[:, sl2], y_sb[:, :])


_NC_CACHE = {}


def _get_nc(N, CORES):
    key = (N, CORES)
    if key not in _NC_CACHE:
        _NC_CACHE[key] = build(N, CORES)
    return _NC_CACHE[key]


def make_win(w1, w2_1, fmax):
    """Pack [w1T | wfT | ep-row] into one [CIN, WIN] fp32 tensor."""
    wf = (w2_1 @ w1)[0]
    win = np.zeros((CIN, WIN), np.float32)
    win[:, 0:COUT] = w1.T
    win[:, COUT] = wf
    win[0, COUT + 1:COUT + 5] = [-fmax, 2.0 * fmax / NE, fmax,
                                 -2.0 * fmax / NE]
    return win


def _numpy_fallback(x, bias_mat, w1, w2_1):
    x2 = x[0].astype(np.float64)
    seq = w1.astype(np.float64) @ x2
    f = (w2_1.astype(np.float64) @ seq)[0]
    logits = f[:, None] + f[None, :]
    lr = np.where(logits >= 0, logits, 0.01 * logits) + bias_mat.astype(np.float64)
    e = np.exp(lr - lr.max(axis=0, keepdims=True))
    coefs = e / e.sum(axis=0, keepdims=True)
    ret = np.einsum('ij,oj->oi', coefs, seq)
    out = np.where(ret > 0, ret, np.exp(np.minimum(ret, 0)) - 1)
    return out[None].astype(np.float32)


def kernel(x, bias_mat, w1, w2_1, **_ignored):
    x = np.ascontiguousarray(np.asarray(x, dtype=np.float32))
    w1 = np.ascontiguousarray(np.asarray(w1, dtype=np.float32))
    w2_1 = np.ascontiguousarray(np.asarray(w2_1, dtype=np.float32))
    bias_mat = np.asarray(bias_mat)
    if bias_mat.size and np.any(bias_mat):
        return _numpy_fallback(x, bias_mat, w1, w2_1)
    B, cin, N = x.shape
    assert B == 1 and cin == CIN
    CORES = 8
    IC = N // CORES
    x2 = x[0]

    nc = _get_nc(N, CORES)
    xbf = x2.astype(ml_dtypes.bfloat16)
    wf = (w2_1 @ w1)[0]
    f = wf @ x2
    fmax = float(np.abs(f).max()) * 1.05 + 0.05
    win = make_win(w1, w2_1, fmax)
    in_maps = []
    for c in range(CORES):
        in_maps.append({
            "x": xbf,
            "xI": np.ascontiguousarray(xbf[:, c * IC:(c + 1) * IC]),
            "win": win,
        })
    res = run_bass_kernel_spmd(nc, in_maps, core_ids=list(range(CORES)))
    y = np.concatenate([res.results[c]["y"] for c in range(CORES)], axis=1)
    return y[None].astype(np.float32)


if __name__ == "__main__":
    rng = np.random.default_rng(0)
    N = 8192
    x = rng.standard_normal((1, CIN, N), dtype=np.float32)
    w1 = (rng.standard_normal((COUT, CIN)) / np.sqrt(CIN)).astype(np.float32)
    w2 = (rng.standard_normal((1, COUT)) / np.sqrt(COUT)).astype(np.float32)
    bias = np.zeros((N, N), np.float32)
    y = kernel(x=x, bias_mat=bias, w1=w1, w2_1=w2)
    print("kernel output", y.shape, y.dtype)


# revision 17
# speedup vs baseline: 1.3593x; 1.3593x over previous
"""Trainium2 Bass kernel for nn_Attn_head (GAT attention head, B=1) — v13.

v12 -> v13 (same math, scheduling/overlap):
  - wf computed host-side (already needed for fmax); w1T/wf/ep packed
    into ONE input tensor "win" -> 3 fewer serial DMA_DIRECT2D issues.
  - xs chunk-0 DMA issued first; xI issued after chunk 1.
  - PE warm-keeping dummy matmuls in the three idle windows (startup,
    Ue->dUT handoff, mid-tail) so the HAM clock gate stays at 8/8.
  - chunk-loop engine rebalance: second sft/F_ALL copies on DVE, nf on
    GpSimd; ACT keeps one copy of each + the exps.
  - tail software-pipelined in 4 supersets of 16 j-blocks
    (U3 copy -> D math -> gall -> TG accumulate), U3 matmuls emitted
    one superset ahead so the PE queue never drains.
"""

import sys
import numpy as np

for _p in ("/opt/trn_rl_repo", "/root/.axon_site/_ro/trn_rl_repo"):
    if _p not in sys.path:
        sys.path.insert(0, _p)

import concourse.bacc as bacc
import concourse.bass as bass
import concourse.mybir as mybir
import concourse.tile as tile
import concourse.masks as masks
import ml_dtypes
from concourse.bass_utils import run_bass_kernel_spmd

FP32 = mybir.dt.float32
BF16 = mybir.dt.bfloat16
INT32 = mybir.dt.int32
ALU = mybir.AluOpType
AF = mybir.ActivationFunctionType

CIN = 128
COUT = 64
W = COUT + 1  # sft width per j-block: seq_ftsT cols + (-f) col
JBW = 128     # j-block width (PE contraction tile)
MF = 512      # moving free dim per matmul (one PSUM bank of fp32)
XCH = 1024    # x staging chunk (columns per DMA)
PBB = 4       # preamble j-blocks per PSUM tile
NE = 128      # grid size (one partition tile)
NEX = NE + 8  # +8 sentinel always-true cols (col NE => column sums)
WIN = COUT + 5  # packed win cols: w1T | wfT | ep(4, on row 0)
SCJ = 32      # tail superset size (j-blocks)


def build(N=8192, CORES=8):
    nc = bacc.Bacc("TRN2", target_bir_lowering=False, debug=False,
                   num_devices=CORES)
    IC = N // CORES
    x_d = nc.dram_tensor("x", [CIN, N], BF16, kind="ExternalInput")
    xI_d = nc.dram_tensor("xI", [CIN, IC], BF16, kind="ExternalInput")
    win_d = nc.dram_tensor("win", [CIN, WIN], FP32, kind="ExternalInput")
    y_d = nc.dram_tensor("y", [COUT, IC], FP32, kind="ExternalOutput")

    with tile.TileContext(nc) as tc:
        _build_body(tc, nc, x_d, xI_d, win_d, y_d, N, CORES)
    nc.compile()
    return nc


def _build_body(tc, nc, x_d, xI_d, win_d, y_d, N, CORES):
    from contextlib import ExitStack
    IC = N // CORES
    NJB = N // JBW
    NCH = N // XCH
    CPX = XCH // JBW
    NH = max(IC // MF, 1)
    MFi = min(MF, IC)
    NSC = max(NJB // SCJ, 1)
    SCJi = min(SCJ, NJB)

    ctx = ExitStack()
    with ctx:
        sb = ctx.enter_context(tc.tile_pool(name="sb", bufs=1))
        xpool = ctx.enter_context(tc.tile_pool(name="xpool", bufs=3))
        mbpool = ctx.enter_context(tc.tile_pool(name="mbpool", bufs=1))
        m1pool = ctx.enter_context(tc.tile_pool(name="m1pool", bufs=1))
        eppool = ctx.enter_context(tc.tile_pool(name="eppool", bufs=2))
        ue_ps_pool = ctx.enter_context(
            tc.tile_pool(name="ue_ps", bufs=1, space="PSUM"))
        pre_ps_pool = ctx.enter_context(
            tc.tile_pool(name="pre_ps", bufs=1, space="PSUM"))
        wp_ps_pool = ctx.enter_context(
            tc.tile_pool(name="wp_ps", bufs=1, space="PSUM"))
        fa_ps_pool = ctx.enter_context(
            tc.tile_pool(name="fa_ps", bufs=2, space="PSUM"))
        big_ps_pool = ctx.enter_context(
            tc.tile_pool(name="big_ps", bufs=1, space="PSUM"))
        misc_ps_pool = ctx.enter_context(
            tc.tile_pool(name="misc_ps", bufs=1, space="PSUM"))

        # ---------------- DMA issue order: x chunk 0 first ----------------
        xs_tiles = []
        xs0 = xpool.tile([CIN, XCH], BF16, name="xs0", tag="xs")
        nc.sync.dma_start(xs0[:, :], x_d.ap()[:, 0:XCH])
        xs_tiles.append(xs0)
        win = sb.tile([CIN, WIN], FP32)
        nc.sync.dma_start(win[:, :], win_d.ap())
        if NCH > 1:
            xs1 = xpool.tile([CIN, XCH], BF16, name="xs1", tag="xs")
            nc.sync.dma_start(xs1[:, :], x_d.ap()[:, XCH:2 * XCH])
            xs_tiles.append(xs1)
        xI_sb = xpool.tile([CIN, IC], BF16, name="xI", tag="xi")
        nc.sync.dma_start(xI_sb[:, :], xI_d.ap())

        # ---------------- phase 0: constants & grid ----------------
        ones = sb.tile([128, 128], FP32)
        nc.gpsimd.memset(ones[:, :], 1.0)
        idT = sb.tile([128, 128], BF16)
        masks.make_identity(nc, idT[:, :])
        id2 = sb.tile([128, COUT], BF16)   # stacked double identity
        masks.make_identity(nc, id2[0:COUT, :])
        masks.make_identity(nc, id2[COUT:128, :])
        it_e = sb.tile([128, NE], INT32)
        nc.gpsimd.iota(it_e[:, :], pattern=[[1, NE]], channel_multiplier=0)
        it_p = sb.tile([128, 1], INT32)
        nc.gpsimd.iota(it_p[:, :], pattern=[[128, 1]], channel_multiplier=1)

        # PE warm-up: keep the HAM activity window busy before real work.
        # wp is a dedicated never-recycled PSUM tile so dummies can be
        # sprinkled anywhere without aliasing live accumulations.
        wp = wp_ps_pool.tile([128, 128], FP32, name="wp", tag="wp")
        for wv in range(44):
            nc.tensor.matmul(wp[:, :], idT[:, :], idT[:, :])

        wfull = sb.tile([CIN, W], BF16)
        nc.vector.tensor_copy(wfull[:, 0:COUT], win[:, 0:COUT])
        nc.vector.tensor_scalar(wfull[:, COUT:W], win[:, COUT:COUT + 1],
                                -1.0, None, ALU.mult)
        wf_rep = sb.tile([CIN, 128], BF16)
        nc.vector.tensor_scalar(wf_rep[:, :], ones[:, :],
                                win[:, COUT:COUT + 1], None, ALU.mult)

        epb_ps = misc_ps_pool.tile([128, 4], FP32, name="epb_ps", tag="mm")
        nc.tensor.matmul(epb_ps[:, :], ones[0:1, :], win[0:1, COUT + 1:WIN])
        epb = sb.tile([128, 4], FP32)
        nc.scalar.activation(epb[:, :], epb_ps[:, :], AF.Copy)
        E_bc = sb.tile([128, NEX], BF16)   # E_bc[p,t] = e_t; cols NE.. = -inf
        nc.scalar.activation(E_bc[:, 0:NE], it_e[:, :], AF.Identity,
                             bias=epb[:, 0:1], scale=epb[:, 1:2])
        nc.gpsimd.memset(E_bc[:, NE:NEX], -1.0e38)
        ecol = sb.tile([128, 1], FP32)     # ecol[p] = -e_p
        nc.scalar.activation(ecol[:, :], it_p[:, :], AF.Identity,
                             bias=epb[:, 2:3], scale=epb[:, 3:4])

        # ---------------- F broadcast (own i-shard) ----------------
        F_sb = sb.tile([128, IC], BF16)    # f[i] bcast over partitions
        ab_bc = sb.tile([128, IC], FP32)   # rows 0:64 exp(f), 64:128 exp(.01f)
        for h in range(NH):
            sl = slice(h * MFi, (h + 1) * MFi)
            fi_ps = fa_ps_pool.tile([128, MFi], FP32, name=f"fi{h}", tag="fa")
            nc.tensor.matmul(fi_ps[:, :], wf_rep[:, :], xI_sb[:, sl])
            nc.scalar.activation(F_sb[:, sl], fi_ps[:, :], AF.Copy)
            nc.scalar.activation(ab_bc[0:COUT, sl], fi_ps[0:COUT, :], AF.Exp)
            nc.scalar.activation(ab_bc[COUT:128, sl], fi_ps[COUT:128, :],
                                 AF.Exp, scale=0.01)
        mt2 = sb.tile([128, IC], BF16)     # mt2[p,i] = [e_p <= -f_i]
        nc.vector.tensor_scalar(mt2[:, :], F_sb[:, :], ecol[:, 0:1], None,
                                ALU.is_le)

        # ---------------- preamble chunk loop ----------------
        sft = sb.tile([128, NJB * W], BF16)   # [j_in_block, (JB, o|-f)]
        F_ALL = sb.tile([128, N], BF16)       # f[j] bcast over partitions
        nf = sb.tile([128, NJB], FP32)        # -f[j]
        a_all = sb.tile([128, NJB], FP32)
        b_all = sb.tile([128, NJB], FP32)
        pf = sb.tile([128, NJB], FP32)        # +f[j]
        abp = sb.tile([128, 2 * NJB], BF16)   # per jb: [b, a] lhsT cols
        Ue_ps = ue_ps_pool.tile([2, NEX], FP32, name="Ue_ps", tag="ue")
        mb_tiles = []
        m1_tiles = []
        for ch in range(NCH):
            j0 = ch * CPX
            if ch < len(xs_tiles):
                xs = xs_tiles[ch]
            else:
                xs = xpool.tile([CIN, XCH], BF16, name=f"xs{ch}", tag="xs")
                nc.sync.dma_start(xs[:, :],
                                  x_d.ap()[:, ch * XCH:(ch + 1) * XCH])
            # F_ALL chunk: stationary wf_rep, one matmul per 512 cols
            for g in range(XCH // MF):
                fa = fa_ps_pool.tile([128, MF], FP32, name=f"fa{ch}{g}",
                                     tag="fa")
                nc.tensor.matmul(fa[:, :], wf_rep[:, :],
                                 xs[:, g * MF:(g + 1) * MF])
                dst = F_ALL[:, ch * XCH + g * MF:ch * XCH + (g + 1) * MF]
                if g == 0:
                    nc.scalar.activation(dst, fa[:, :], AF.Copy)
                else:
                    nc.vector.tensor_copy(dst, fa[:, :])
            # m1 chunk: m1[p, j] = [e_p <= -f_j]
            m1c = m1pool.tile([128, XCH], BF16, name=f"m1_{ch}", tag=f"m1{ch}")
            nc.vector.tensor_scalar(
                m1c[:, :], F_ALL[:, ch * XCH:(ch + 1) * XCH], ecol[:, 0:1],
                None, ALU.is_le)
            m1_tiles.append(m1c)
            # seq_ftsT + (-f) per PBB-group
            for g in range(CPX // PBB):
                jg = j0 + g * PBB
                pre = pre_ps_pool.tile([128, PBB * W], FP32, name=f"pre{jg}",
                                       tag="pre")
                for k in range(PBB):
                    xo = (g * PBB + k) * JBW
                    nc.tensor.matmul(pre[:, k * W:(k + 1) * W],
                                     xs[:, xo:xo + JBW], wfull[:, :])
                dst = sft[:, jg * W:(jg + PBB) * W]
                if g == 0:
                    nc.scalar.activation(dst, pre[:, :], AF.Copy)
                else:
                    nc.vector.tensor_copy(dst, pre[:, :])
            csl = slice(j0, j0 + CPX)
            nc.scalar.activation(
                nf[:, csl], sft[:, j0 * W + COUT:(j0 + CPX) * W:W], AF.Copy)
            nc.scalar.activation(a_all[:, csl], nf[:, csl], AF.Exp, scale=-1.0)
            nc.scalar.activation(b_all[:, csl], nf[:, csl], AF.Exp,
                                 scale=-0.01)
            nc.gpsimd.tensor_scalar(pf[:, csl], nf[:, csl], -1.0, None,
                                    ALU.mult)
            nc.gpsimd.tensor_copy(abp[:, 2 * j0 + 0:2 * (j0 + CPX):2],
                                  b_all[:, csl])
            nc.gpsimd.tensor_copy(abp[:, 2 * j0 + 1:2 * (j0 + CPX):2],
                                  a_all[:, csl])
            for jb in range(j0, j0 + CPX):
                mb = mbpool.tile([128, NEX], BF16, name=f"mb{jb}",
                                 tag=f"mb{jb}")
                nc.vector.tensor_scalar(mb[:, :], E_bc[:, :],
                                        pf[:, jb:jb + 1], None, ALU.is_le)
                nc.tensor.matmul(Ue_ps[:, :], abp[:, 2 * jb:2 * jb + 2],
                                 mb[:, :],
                                 start=(jb == 0), stop=(jb == NJB - 1))
                mb_tiles.append(mb)
            for wv in range(2):
                nc.tensor.matmul(wp[:, :], idT[:, :], idT[:, :])

        # ---------------- U_e -> dU; PE kept warm over the handoff --------
        Ue = sb.tile([2, NEX], FP32)
        nc.scalar.activation(Ue[:, :], Ue_ps[:, :], AF.Copy)
        for wv in range(10):
            nc.tensor.matmul(wp[:, :], idT[:, :], idT[:, :])
        sb_ps = misc_ps_pool.tile([128, 1], FP32, name="sb_ps", tag="mm")
        nc.tensor.matmul(sb_ps[:, :], ones[0:1, :], Ue[0:1, 0:1])
        Sb_bc = sb.tile([128, 1], FP32)
        nc.scalar.activation(Sb_bc[:, :], sb_ps[:, :], AF.Copy)
        dU = sb.tile([2, NE], BF16)
        nc.vector.tensor_tensor(dU[:, 1:NE - 1], Ue[:, 2:NE], Ue[:, 1:NE - 1],
                                ALU.subtract)
        nc.vector.tensor_scalar(dU[:, NE - 1:NE], Ue[:, NE - 1:NE], -1.0,
                                None, ALU.mult)
        nc.vector.tensor_scalar(dU[:, 0:1], Ue[:, 1:2], 1.0, None, ALU.mult)
        dUT_ps = misc_ps_pool.tile([128, 2], BF16, name="dUT_ps", tag="mm")
        nc.tensor.transpose(dUT_ps[:, :], dU[:, :], idT[0:2, 0:2])
        dUT = sb.tile([128, 2], BF16)
        nc.scalar.activation(dUT[:, :], dUT_ps[:, :], AF.Copy)

        # ------- tail: supersets of SCJ j-blocks, pipelined by one -------
        U3_ps = misc_ps_pool.tile([128, 2 * NJB], FP32, name="U3", tag="mm")
        U3 = sb.tile([128, 2 * NJB], FP32)
        aDb = sb.tile([128, NJB], FP32)
        bDb = sb.tile([128, NJB], FP32)
        t1 = sb.tile([128, NJB], FP32)
        s2 = sb.tile([128, NJB], FP32)
        t2 = sb.tile([128, NJB], FP32)
        D_T = sb.tile([128, NJB], FP32)
        Dinv = sb.tile([128, NJB], FP32)
        TG_ps = ue_ps_pool.tile([128, NEX], FP32, name="TG_ps", tag="ue")
        gtiles = {}

        def emit_u3(sc):
            for jb in range(sc * SCJi, (sc + 1) * SCJi):
                nc.tensor.matmul(
                    U3_ps[:, 2 * jb:2 * jb + 2],
                    m1_tiles[jb // CPX][:, (jb % CPX) * JBW:
                                        (jb % CPX + 1) * JBW],
                    dUT[:, :])

        def emit_dgal(sc):
            jsl = slice(sc * SCJi, (sc + 1) * SCJi)
            u2sl = slice(2 * sc * SCJi, 2 * (sc + 1) * SCJi)
            nc.scalar.activation(U3[:, u2sl], U3_ps[:, u2sl], AF.Copy)
            for wv in range(4):
                nc.tensor.matmul(wp[:, :], idT[:, :], idT[:, :])
            nc.vector.tensor_tensor(t1[:, jsl], a_all[:, jsl],
                                    U3[:, u2sl][:, 1::2], ALU.mult)
            nc.vector.tensor_scalar(s2[:, jsl], U3[:, u2sl][:, 0::2],
                                    Sb_bc[:, 0:1], -1.0,
                                    ALU.subtract, ALU.mult)
            nc.vector.tensor_tensor(t2[:, jsl], b_all[:, jsl], s2[:, jsl],
                                    ALU.mult)
            nc.vector.tensor_tensor(D_T[:, jsl], t1[:, jsl], t2[:, jsl],
                                    ALU.add)
            nc.vector.reciprocal(Dinv[:, jsl], D_T[:, jsl])
            nc.vector.tensor_tensor(aDb[:, jsl], a_all[:, jsl], Dinv[:, jsl],
                                    ALU.mult)
            nc.vector.tensor_tensor(bDb[:, jsl], b_all[:, jsl], Dinv[:, jsl],
                                    ALU.mult)
            for c in range(sc * SCJi // CPX,
                           max((sc + 1) * SCJi // CPX, sc * SCJi // CPX + 1)):
                j0 = c * CPX
                gpx = min(CPX, NJB - j0)
                gc = sb.tile([128, gpx * 2 * COUT], BF16, name=f"gall{c}")
                gvv = gc[:, :].rearrange("p (j t) -> p j t", t=2 * COUT)
                sfv = sft[:, j0 * W:(j0 + gpx) * W].rearrange(
                    "p (j w) -> p j w", w=W)[:, :, 0:COUT]
                nc.vector.tensor_tensor(
                    gvv[:, :, 0:COUT], sfv,
                    aDb[:, j0:j0 + gpx].unsqueeze(2).broadcast_to(
                        [128, gpx, COUT]), ALU.mult)
                nc.gpsimd.tensor_tensor(
                    gvv[:, :, COUT:2 * COUT], sfv,
                    bDb[:, j0:j0 + gpx].unsqueeze(2).broadcast_to(
                        [128, gpx, COUT]), ALU.mult)
                gtiles[c] = gc

        def emit_tg(sc):
            for jb in range(sc * SCJi, (sc + 1) * SCJi):
                gc = gtiles[jb // CPX]
                go = (jb % CPX) * 2 * COUT
                nc.tensor.matmul(TG_ps[:, :], gc[:, go:go + 2 * COUT],
                                 mb_tiles[jb][:, :],
                                 start=(jb == 0), stop=(jb == NJB - 1))

        emit_u3(0)
        if NSC > 1:
            emit_dgal(0)
            emit_u3(1)
            for sc in range(2, NSC):
                emit_tg(sc - 2)
                emit_dgal(sc - 1)
                emit_u3(sc)
            emit_tg(NSC - 2)
            emit_dgal(NSC - 1)
            emit_tg(NSC - 1)
        else:
            emit_dgal(0)
            emit_tg(0)

        sgb_col = sb.tile([128, 1], FP32)   # rows 64:128 = SGb
        nc.scalar.activation(sgb_col[COUT:128, :], TG_ps[COUT:128, NE:NE + 1],
                             AF.Copy)
        TGs = sb.tile([128, NEX], FP32)
        nc.scalar.activation(TGs[:, :], TG_ps[:, :], AF.Copy)
        for wv in range(6):
            nc.tensor.matmul(wp[:, :], idT[:, :], idT[:, :])
        dTG = sb.tile([128, NE], BF16)
        nc.vector.tensor_tensor(dTG[:, 1:NE - 1], TGs[:, 2:NE],
                                TGs[:, 1:NE - 1], ALU.subtract)
        nc.vector.tensor_scalar(dTG[:, NE - 1:NE], TGs[:, NE - 1:NE], -1.0,
                                None, ALU.mult)
        nc.vector.tensor_scalar(dTG[:, 0:1], TGs[:, 1:2], 1.0, None, ALU.mult)
        dTGT_ps = misc_ps_pool.tile([128, 128], BF16, name="dTGT_ps", tag="mm")
        nc.tensor.transpose(dTGT_ps[:, :], dTG[:, :], idT[:, :])
        dTGT = sb.tile([128, 128], BF16)
        nc.scalar.activation(dTGT[:, :], dTGT_ps[:, :], AF.Copy)

        # ---------------- gather + epilogue per column half ----------------
        out_ps = big_ps_pool.tile([128, IC], FP32, name="out_ps", tag="big")
        for h2 in range(NH):
            sl2 = slice(h2 * MFi, (h2 + 1) * MFi)
            nc.tensor.matmul(out_ps[:, sl2], dTGT[:, :], mt2[:, sl2])
            tfu = eppool.tile([128, MFi], BF16, name=f"tf{h2}", tag="e1")
            nc.vector.tensor_tensor(tfu[0:COUT, :], ab_bc[0:COUT, sl2],
                                    out_ps[0:COUT, sl2], ALU.mult)
            eb = eppool.tile([128, MFi], FP32, name=f"eb{h2}", tag="e2")
            nc.scalar.activation(eb[COUT:128, :], out_ps[COUT:128, sl2],
                                 AF.Identity, bias=sgb_col[COUT:128, 0:1],
                                 scale=-1.0)
            nc.vector.tensor_tensor(tfu[COUT:128, :], ab_bc[COUT:128, sl2],
                                    eb[COUT:128, :], ALU.mult)
            z_ps = misc_ps_pool.tile([COUT, MFi], FP32, name=f"z{h2}",
                                     tag="mm")
            nc.tensor.matmul(z_ps[:, :], id2[:, :], tfu[:, :])
            e = eppool.tile([COUT, MFi], BF16, name=f"e{h2}", tag="e3")
            nc.scalar.activation(e[:, :], z_ps[:, :], AF.Exp)
            r = eppool.tile([COUT, MFi], BF16, name=f"r{h2}", tag="e4")
            nc.scalar.activation(r[:, :], z_ps[:, :], AF.Relu)
            q = eppool.tile([COUT, MFi], BF16, name=f"q{h2}", tag="e5")
            nc.vector.tensor_scalar(q[:, :], e[:, :], 1.0, -1.0, ALU.min,
                                    ALU.add)
            y_sb = eppool.tile([COUT, MFi], FP32, name=f"y{h2}", tag="e6")
            nc.vector.tensor_tensor(y_sb[:, :], r[:, :], q[:, :], ALU.add)
            nc.sync.dma_start(y_d.ap()[:, sl2], y_sb[:, :])


_NC_CACHE = {}


def _get_nc(N, CORES):
    key = (N, CORES)
    if key not in _NC_CACHE:
        _NC_CACHE[key] = build(N, CORES)
    return _NC_CACHE[key]


def make_win(w1, w2_1, fmax):
    """Pack [w1T | wfT | ep-row] into one [CIN, WIN] fp32 tensor."""
    wf = (w2_1 @ w1)[0]
    win = np.zeros((CIN, WIN), np.float32)
    win[:, 0:COUT] = w1.T
    win[:, COUT] = wf
    win[0, COUT + 1:COUT + 5] = [-fmax, 2.0 * fmax / NE, fmax,
                                 -2.0 * fmax / NE]
    return win


def _numpy_fallback(x, bias_mat, w1, w2_1):
    x2 = x[0].astype(np.float64)
    seq = w1.astype(np.float64) @ x2
    f = (w2_1.astype(np.float64) @ seq)[0]
    logits = f[:, None] + f[None, :]
    lr = np.where(logits >= 0, logits, 0.01 * logits) + bias_mat.astype(np.float64)
    e = np.exp(lr - lr.max(axis=0, keepdims=True))
    coefs = e / e.sum(axis=0, keepdims=True)
    ret = np.einsum('ij,oj->oi', coefs, seq)
    out = np.where(ret > 0, ret, np.exp(np.minimum(ret, 0)) - 1)
    return out[None].astype(np.float32)


def kernel(x, bias_mat, w1, w2_1, **_ignored):
    x = np.ascontiguousarray(np.asarray(x, dtype=np.float32))
    w1 = np.ascontiguousarray(np.asarray(w1, dtype=np.float32))
    w2_1 = np.ascontiguousarray(np.asarray(w2_1, dtype=np.float32))
    bias_mat = np.asarray(bias_mat)
    if bias_mat.size and np.any(bias_mat):
        return _numpy_fallback(x, bias_mat, w1, w2_1)
    B, cin, N = x.shape
    assert B == 1 and cin == CIN
    CORES = 8
    IC = N // CORES
    x2 = x[0]

    nc = _get_nc(N, CORES)
    xbf = x2.astype(ml_dtypes.bfloat16)
    wf = (w2_1 @ w1)[0]
    f = wf @ x2
    fmax = float(np.abs(f).max()) * 1.05 + 0.05
    win = make_win(w1, w2_1, fmax)
    in_maps = []
    for c in range(CORES):
        in_maps.append({
            "x": xbf,
            "xI": np.ascontiguousarray(xbf[:, c * IC:(c + 1) * IC]),
            "win": win,
        })
    res = run_bass_kernel_spmd(nc, in_maps, core_ids=list(range(CORES)))
    y = np.concatenate([res.results[c]["y"] for c in range(CORES)], axis=1)
    return y[None].astype(np.float32)


if __name__ == "__main__":
    rng = np.random.default_rng(0)
    N = 8192
    x = rng.standard_normal((1, CIN, N), dtype=np.float32)
    w1 = (rng.standard_normal((COUT, CIN)) / np.sqrt(CIN)).astype(np.float32)
    w2 = (rng.standard_normal((1, COUT)) / np.sqrt(COUT)).astype(np.float32)
    bias = np.zeros((N, N), np.float32)
    y = kernel(x=x, bias_mat=bias, w1=w1, w2_1=w2)
    print("kernel output", y.shape, y.dtype)
